# revision 1
# baseline (speedup 1.0000x reference)
import os
import numpy as np

# Problem constants (hardcoded; kernel.py must be self-contained)
N, D, T, K, P = 4000, 256, 52, 20, 100
M = 8            # cores
NS = N // M      # 500 patients per core
NB = 125         # patient sub-block (partition dim), 4 per core
NNB = NS // NB   # 4


def _make_kernel_mat(length_scale):
    t = np.arange(T, dtype=np.float32)
    sq = (t[None, :] - t[:, None]) ** 2
    Kmat = np.exp(-0.5 * sq / np.float32(length_scale) ** 2).astype(np.float32)
    jitter = 1e-4
    eye = np.eye(T, dtype=np.float32)
    while True:
        if np.linalg.cond(Kmat + jitter * eye) < 1e4:
            break
        jitter *= 2
        if jitter > 0.1:
            break
    return (Kmat + jitter * eye).astype(np.float32)


_KINV_LAM = np.linalg.inv(_make_kernel_mat(T / 4).astype(np.float64))
_KINV_PHI = np.linalg.inv(_make_kernel_mat(T / 3).astype(np.float64))

_COMPILED = {}


def _build_nc():
    import concourse.bass as bass
    import concourse.mybir as mybir
    from concourse import bacc, tile

    fp32 = mybir.dt.float32
    Alu = mybir.AluOpType
    Act = mybir.ActivationFunctionType

    nc = bacc.Bacc(None, target_bir_lowering=False)
    lam_tkn = nc.dram_tensor("lam_tkn", [T, K, NS], fp32, kind="ExternalInput")
    lam_nk = nc.dram_tensor("lam_nk", [NS, K, T], fp32, kind="ExternalInput")
    y_tnd = nc.dram_tensor("y_tnd", [T, NS, D], fp32, kind="ExternalInput")
    ef_d = nc.dram_tensor("ef", [NS, D], fp32, kind="ExternalInput")
    phi_tkd = nc.dram_tensor("phi_tkd", [T, K, D + 1], fp32, kind="ExternalInput")
    phid = nc.dram_tensor("phid", [K * D, T], fp32, kind="ExternalInput")
    lp_d = nc.dram_tensor("lp", [D, T], fp32, kind="ExternalInput")
    gtg_d = nc.dram_tensor("gtg", [P, NS + K], fp32, kind="ExternalInput")

    o_data = nc.dram_tensor("o_data", [NB, T * NNB], fp32, kind="ExternalOutput")
    o_slam = nc.dram_tensor("o_slam", [T, T], fp32, kind="ExternalOutput")
    o_sphi = nc.dram_tensor("o_sphi", [T, T], fp32, kind="ExternalOutput")

    with tile.TileContext(nc) as tc:
        with (
            tc.tile_pool(name="res", bufs=1) as res,
            tc.tile_pool(name="efp", bufs=1) as efp,
            tc.tile_pool(name="ld", bufs=3) as ld,
            tc.tile_pool(name="wk", bufs=3) as wk,
            tc.tile_pool(name="yp", bufs=4) as yp,
            tc.tile_pool(name="gp", bufs=3) as gpp,
            tc.tile_pool(name="ps", bufs=3, space=bass.MemorySpace.PSUM) as ps,
            tc.tile_pool(name="psg", bufs=1, space=bass.MemorySpace.PSUM) as psg,
            tc.tile_pool(name="psm", bufs=4, space=bass.MemorySpace.PSUM) as psm,
        ):
            # ---------- resident small tensors ----------
            gtg_ld = res.tile([P, NS + K], fp32, tag="gtgld")
            nc.sync.dma_start(gtg_ld[:], gtg_d[:])
            gtg_sb = res.tile([P, NS + K], fp32, tag="gtg")
            nc.vector.tensor_scalar(out=gtg_sb[:], in0=gtg_ld[:], scalar1=0.0,
                                    scalar2=None, op0=Alu.add)
            gt_sb = gtg_sb[:, :NS]
            gam_sb = gtg_sb[:, NS:NS + K]
            lp_sb = [res.tile([128, T], fp32, tag=f"lp{h}", name=f"lp{h}")
                     for h in range(2)]
            for h in range(2):
                nc.sync.dma_start(lp_sb[h][:], lp_d[h * 128:(h + 1) * 128, :])
            ef_sb = [efp.tile([NB, D], fp32, tag=f"ef{b}", name=f"ef{b}")
                     for b in range(NNB)]
            for b in range(NNB):
                nc.sync.dma_start(ef_sb[b][:], ef_d[b * NB:(b + 1) * NB, :])

            o_acc = res.tile([NB, T * NNB], fp32, tag="oacc")
            nc.vector.memset(o_acc[:], 0.0)

            # ---------- GP phi: S_phi = dev^T dev over 40 row blocks ----------
            sphi_sb = res.tile([T, T], fp32, tag="sphisb")
            nc.vector.memset(sphi_sb[:], 0.0)
            nblk = (K * D) // 128  # 40
            for i in range(nblk):
                phd = gpp.tile([128, T], fp32, tag="phd")
                nc.sync.dma_start(phd[:], phid[i * 128:(i + 1) * 128, :])
                devp = gpp.tile([128, T], fp32, tag="devp")
                nc.vector.scalar_tensor_tensor(
                    out=devp[:], in0=phd[:], scalar=1.0, in1=lp_sb[i % 2][:],
                    op0=Alu.mult, op1=Alu.subtract)
                gram = psg.tile([T, T], fp32, tag="gram")
                nc.tensor.matmul(gram[:], devp[:], devp[:])
                nc.vector.scalar_tensor_tensor(
                    out=sphi_sb[:], in0=sphi_sb[:], scalar=1.0, in1=gram[:],
                    op0=Alu.mult, op1=Alu.add)
            nc.sync.dma_start(o_sphi[:], sphi_sb[:])

            # ---------- GP lambda ----------
            slam_sb = res.tile([T, T], fp32, tag="slamsb")
            nc.vector.memset(slam_sb[:], 0.0)
            for b in range(NNB):
                mean_ps = psm.tile([NB, K], fp32, tag="mean")
                nc.tensor.matmul(mean_ps[:], gt_sb[:, b * NB:(b + 1) * NB],
                                 gam_sb[:, :])
                mean_sb = wk.tile([NB, K], fp32, tag="meansb")
                nc.scalar.activation(mean_sb[:], mean_ps[:], Act.Copy)
                for k in range(K):
                    lamt = gpp.tile([NB, T], fp32, tag="lamt")
                    nc.sync.dma_start(lamt[:], lam_nk[b * NB:(b + 1) * NB, k, :])
                    devl = gpp.tile([NB, T], fp32, tag="devl")
                    nc.vector.tensor_scalar(
                        out=devl[:], in0=lamt[:], scalar1=mean_sb[:, k:k + 1],
                        scalar2=None, op0=Alu.subtract)
                    gram = psg.tile([T, T], fp32, tag="gram")
                    nc.tensor.matmul(gram[:], devl[:], devl[:])
                    nc.vector.scalar_tensor_tensor(
                        out=slam_sb[:], in0=slam_sb[:], scalar=1.0,
                        in1=gram[:], op0=Alu.mult, op1=Alu.add)
            nc.sync.dma_start(o_slam[:], slam_sb[:])

            # ---------- data loss ----------
            for t in range(T):
                lam_t = ld.tile([K, NS], fp32, tag="lamt_e")
                nc.sync.dma_start(lam_t[:], lam_tkn[t, :, :])
                e_t = ld.tile([K, NS], fp32, tag="et")
                nc.scalar.activation(e_t[:], lam_t[:], Act.Exp)
                phi_t = ld.tile([K, D + 1], fp32, tag="phit")
                nc.sync.dma_start(phi_t[:], phi_tkd[t, :, :])
                phie_t = ld.tile([K, D + 1], fp32, tag="phiet")
                nc.scalar.activation(phie_t[:], phi_t[:], Act.Sigmoid)

                for b in range(NNB):
                    y_t = yp.tile([NB, D], fp32, tag="yt")
                    nc.sync.dma_start(y_t[:], y_tnd[t, b * NB:(b + 1) * NB, :])
                    praw = ps.tile([NB, D + 1], fp32, tag="praw")
                    nc.tensor.matmul(praw[:], e_t[:, b * NB:(b + 1) * NB],
                                     phie_t[:])
                    praw_sb = wk.tile([NB, D + 1], fp32, tag="prawsb")
                    nc.scalar.activation(praw_sb[:], praw[:], Act.Copy)
                    r_sb = wk.tile([NB, 1], fp32, tag="rsb")
                    nc.vector.reciprocal(r_sb[:], praw_sb[:, D:D + 1])
                    # u = (ef==t)*Y
                    u = wk.tile([NB, D], fp32, tag="u")
                    nc.vector.scalar_tensor_tensor(
                        out=u[:], in0=ef_sb[b][:], scalar=float(t), in1=y_t[:],
                        op0=Alu.is_equal, op1=Alu.mult)
                    # mle = (ef>=t)
                    mle = wk.tile([NB, D], fp32, tag="mle")
                    nc.vector.tensor_scalar(
                        out=mle[:], in0=ef_sb[b][:], scalar1=float(t),
                        scalar2=None, op0=Alu.is_ge)
                    # w = mle - 2u
                    w = wk.tile([NB, D], fp32, tag="w")
                    nc.vector.scalar_tensor_tensor(
                        out=w[:], in0=u[:], scalar=-2.0, in1=mle[:],
                        op0=Alu.mult, op1=Alu.add)
                    # t1 = (w * r) * praw = w * pi
                    t1 = wk.tile([NB, D], fp32, tag="t1")
                    nc.vector.scalar_tensor_tensor(
                        out=t1[:], in0=w[:], scalar=r_sb[:, 0:1], in1=praw_sb[:, :D],
                        op0=Alu.mult, op1=Alu.mult)
                    # s = -u - t1  (so x = 1 + s)
                    s = wk.tile([NB, D], fp32, tag="s")
                    nc.vector.scalar_tensor_tensor(
                        out=s[:], in0=u[:], scalar=-1.0, in1=t1[:],
                        op0=Alu.mult, op1=Alu.subtract)
                    # log(1 + s), accumulate sum into o_acc column
                    xl = wk.tile([NB, D], fp32, tag="xl")
                    col = t * NNB + b
                    nc.scalar.activation(xl[:], s[:], Act.Ln, bias=1.0,
                                         accum_out=o_acc[:, col:col + 1])
            nc.sync.dma_start(o_data[:], o_acc[:])
    if not nc.is_finalized():
        nc.finalize()
    return nc


def kernel(lambda_, phi, gamma, G, Y, logit_prev_t, event_times):
    from concourse.bass_utils import run_bass_kernel_spmd

    lambda_ = np.asarray(lambda_, dtype=np.float32)
    phi = np.asarray(phi, dtype=np.float32)
    gamma = np.asarray(gamma, dtype=np.float32)
    G = np.asarray(G, dtype=np.float32)
    Y = np.asarray(Y, dtype=np.float32)
    logit_prev_t = np.asarray(logit_prev_t, dtype=np.float32)
    ef_all = np.asarray(event_times).astype(np.float32)

    if "nc" not in _COMPILED:
        _COMPILED["nc"] = _build_nc()
    nc = _COMPILED["nc"]

    phi_pad = np.concatenate(
        [phi.transpose(2, 0, 1), np.full((T, K, 1), 20.0, np.float32)], axis=2)
    phi_tkd = np.ascontiguousarray(phi_pad)
    phid = np.ascontiguousarray(phi.reshape(K * D, T))
    in_maps = []
    for c in range(M):
        sl = slice(c * NS, (c + 1) * NS)
        in_maps.append(dict(
            lam_tkn=np.ascontiguousarray(lambda_[sl].transpose(2, 1, 0)),
            lam_nk=np.ascontiguousarray(lambda_[sl]),
            y_tnd=np.ascontiguousarray(Y[sl].transpose(2, 0, 1)),
            ef=np.ascontiguousarray(ef_all[sl]),
            phi_tkd=phi_tkd,
            phid=phid,
            lp=logit_prev_t,
            gtg=np.ascontiguousarray(
                np.concatenate([G[sl].T, gamma], axis=1)),
        ))

    res = run_bass_kernel_spmd(nc, in_maps, list(range(M)))
    data_sum = 0.0
    q_lam = 0.0
    for c in range(M):
        data_sum += float(res.results[c]["o_data"].sum(dtype=np.float64))
        q_lam += float((_KINV_LAM * res.results[c]["o_slam"].astype(np.float64)).sum())
    q_phi = float((_KINV_PHI * res.results[0]["o_sphi"].astype(np.float64)).sum())

    loss = -data_sum / N + 0.5 * q_lam / N + 0.5 * q_phi / D
    return np.array(loss, dtype=np.float32)



# revision 17
# speedup vs baseline: 2.5022x; 2.5022x over previous
import numpy as np
import ml_dtypes

# Problem constants (hardcoded; kernel.py must be self-contained)
N, D, T, K, P = 4000, 256, 52, 20, 100
M = 8            # cores
NS = N // M      # 500 patients per core
KP = 32          # K padded to 32 so each t-group stays inside one partition tile
NBLK = (T * KP) // 128   # 13 blocks of 128 (t,k) rows
DC = 2           # d-chunks of 128
HALF_G = (7, 6)  # 4t-groups per half (7*4=28 t, 6*4=24 t)

BF16 = ml_dtypes.bfloat16


def _make_kernel_mat(length_scale):
    t = np.arange(T, dtype=np.float32)
    sq = (t[None, :] - t[:, None]) ** 2
    Kmat = np.exp(-0.5 * sq / np.float32(length_scale) ** 2).astype(np.float32)
    jitter = 1e-4
    eye = np.eye(T, dtype=np.float32)
    while True:
        if np.linalg.cond(Kmat + jitter * eye) < 1e4:
            break
        jitter *= 2
        if jitter > 0.1:
            break
    return (Kmat + jitter * eye).astype(np.float32)


_KINV_LAM = np.linalg.inv(_make_kernel_mat(T / 4).astype(np.float64))
_KINV_PHI = np.linalg.inv(_make_kernel_mat(T / 3).astype(np.float64))

_COMPILED = {}


def _build_nc():
    import os
    import concourse.bass as bass
    import concourse.mybir as mybir
    from concourse import bacc, tile

    use_paged = os.environ.get("KPAGED", "0") == "1"
    use_fastr = os.environ.get("KFASTR", "0") == "1"
    use_masktt = os.environ.get("KMASKTT", "0") == "1"

    fp32 = mybir.dt.float32
    bf16 = mybir.dt.bfloat16
    Alu = mybir.AluOpType
    Act = mybir.ActivationFunctionType

    nc = bacc.Bacc(None, target_bir_lowering=False)

    # ---- DRAM inputs (host-prepacked layouts) ----
    lam32_d = nc.dram_tensor("lam32", [128, NBLK * NS], bf16, kind="ExternalInput")
    phi32_d = nc.dram_tensor("phi32", [128, NBLK * D], bf16, kind="ExternalInput")
    efT_d = nc.dram_tensor("efT", [128, DC * NS], bf16, kind="ExternalInput")
    yeT_d = nc.dram_tensor("yeT", [128, DC * NS], bf16, kind="ExternalInput")
    # lam rows interleaved with one-hole-per-k for the device-written mean
    lamg_d = nc.dram_tensor("lamg", [125, 4 * K * (T + 1)], bf16,
                            kind="ExternalInput")
    gtg_d = nc.dram_tensor("gtg", [P, NS + K], bf16, kind="ExternalInput")
    # [phi_row | lp_row] pairs for the fused phi gram
    phig2_d = nc.dram_tensor("phig2", [128, 40 * 2 * T], bf16,
                             kind="ExternalInput")
    # event-time one-hot masks (t-major), one 26000-wide block per chunk
    b_d = nc.dram_tensor("bmask", [128, DC * T * NS], bf16, kind="ExternalInput")
    if use_masktt:
        # interleaved [a_h0|b_h0|a_h1|b_h1] per chunk
        masks_d = nc.dram_tensor("masks", [128, DC * 2 * T * NS], bf16,
                                 kind="ExternalInput")
    # scratch for the 1/R broadcast bounce
    rr16_d = nc.dram_tensor("rr16scr", [T, NS], bf16, kind="Internal")

    # ---- DRAM outputs ----
    o_dacc = nc.dram_tensor("o_dacc", [128, 8], fp32, kind="ExternalOutput")
    o_glam = nc.dram_tensor("o_glam", [T + 1, T + 1], fp32, kind="ExternalOutput")
    o_gphi = nc.dram_tensor("o_gphi", [2 * T, 2 * T], fp32, kind="ExternalOutput")

    with tile.TileContext(nc) as tc:
        with (
            tc.tile_pool(name="res", bufs=1) as res,
            tc.tile_pool(name="scr", bufs=4) as scr,
            tc.tile_pool(name="cpx", bufs=2) as cpx,
        ):
            efT = res.tile([128, DC * NS], bf16, tag="efT")
            nc.sync.dma_start(efT[:], efT_d[:])
            yeT = res.tile([128, DC * NS], bf16, tag="yeT")
            nc.sync.dma_start(yeT[:], yeT_d[:])
            theta = res.tile([128, NBLK * NS], bf16, tag="theta")
            phibar = res.tile([128, NBLK * D], bf16, tag="phibar")
            ones32 = res.tile([128, KP], bf16, tag="ones32")
            nc.vector.memset(ones32[:], 1.0)
            dacc = res.tile([128, 8], fp32, tag="dacc")
            nc.vector.memset(dacc[:], 0.0)

            with tc.tile_pool(name="setup", bufs=1) as setup:
                lam32 = setup.tile([128, NBLK * NS], bf16, tag="lam32")
                nc.sync.dma_start(lam32[:], lam32_d[:])
                phi32 = setup.tile([128, NBLK * D], bf16, tag="phi32")
                nc.sync.dma_start(phi32[:], phi32_d[:])
                lamg = setup.tile([125, 4 * K * (T + 1)], bf16, tag="lamg")
                nc.sync.dma_start(lamg[:], lamg_d[:])
                gtg = setup.tile([P, NS + K], bf16, tag="gtg")
                nc.sync.dma_start(gtg[:], gtg_d[:])
                phig2 = setup.tile([128, 40 * 2 * T], bf16, tag="phig2")
                nc.sync.dma_start(phig2[:], phig2_d[:])
                gout = setup.tile([2 * T, 2 * T + T + 1], fp32, tag="gout")

                # ===== GP phase: mean, then fused grams =====
                with tc.tile_pool(name="gps", bufs=1,
                                  space=bass.MemorySpace.PSUM) as gps:
                    mean_ps = gps.tile([125, 512], fp32, tag="mean_ps")
                    for b in range(4):
                        nc.tensor.matmul(mean_ps[:, 0:K],
                                         gtg[:, b * 125:(b + 1) * 125],
                                         gtg[:, NS:NS + K])
                        # write mean into the per-(b,k) hole at slot offset 52
                        hole = lamg[:, :].rearrange(
                            "p (s w) -> p s w", w=T + 1)[:, b * K:(b + 1) * K,
                                                         T:T + 1]
                        nc.scalar.activation(hole, mean_ps[:, 0:K].unsqueeze(2),
                                             Act.Copy)

                    glam_ps = gps.tile([T + 1, 512], fp32, tag="glam_ps")
                    nmm = 4 * K
                    for i in range(nmm):
                        v = lamg[:, i * (T + 1):(i + 1) * (T + 1)]
                        nc.tensor.matmul(glam_ps[:, 0:T + 1], v, v,
                                         start=(i == 0), stop=(i == nmm - 1),
                                         skip_group_check=True)
                    gphi_ps = gps.tile([2 * T, 512], fp32, tag="gphi_ps")
                    for i in range(40):
                        v = phig2[:, i * 2 * T:(i + 1) * 2 * T]
                        nc.tensor.matmul(gphi_ps[:, 0:2 * T], v, v,
                                         start=(i == 0), stop=(i == 39),
                                         skip_group_check=True)
                    nc.scalar.activation(gout[0:T + 1, 0:T + 1],
                                         glam_ps[0:T + 1, 0:T + 1], Act.Copy)
                    nc.scalar.activation(gout[:, T + 1:T + 1 + 2 * T],
                                         gphi_ps[:, 0:2 * T], Act.Copy)
                    nc.sync.dma_start(o_glam[:], gout[0:T + 1, 0:T + 1])
                    nc.sync.dma_start(o_gphi[:], gout[:, T + 1:T + 1 + 2 * T])

                # ===== theta = softmax(lambda): e * (1/R) =====
                e32 = setup.tile([128, NBLK * NS], bf16, tag="e32")
                nc.scalar.activation(e32[:], lam32[:], Act.Exp)
                racc = setup.tile([T, NS], fp32, tag="racc")
                with tc.tile_pool(name="rrp", bufs=2,
                                  space=bass.MemorySpace.PSUM) as rrp:
                    for blk in range(NBLK):
                        rrep = rrp.tile([128, 512], fp32, tag="rrep")
                        for pg in range(4):
                            nc.tensor.matmul(
                                rrep[pg * 32:(pg + 1) * 32, 0:NS],
                                ones32[pg * 32:(pg + 1) * 32, :],
                                e32[pg * 32:pg * 32 + KP,
                                    blk * NS:(blk + 1) * NS],
                                skip_group_check=True,
                                tile_position=(pg * 32, pg * 32))
                        if use_fastr:
                            rc = scr.tile([128, NS], fp32, tag="rc")
                            nc.scalar.activation(rc[:], rrep[:, 0:NS],
                                                 Act.Copy)
                            # gather the 4 unique t-rows {0,32,64,96}
                            src = rc[:, :].rearrange(
                                "(a b) n -> a b n", b=32)[:, 0, :]
                            nc.sync.dma_start(racc[4 * blk:4 * blk + 4, :],
                                              src)
                        else:
                            rinv_b = scr.tile([128, NS], fp32, tag="rinvb")
                            nc.vector.reciprocal(rinv_b[:], rrep[:, 0:NS])
                            nc.vector.tensor_tensor(
                                out=theta[:, blk * NS:(blk + 1) * NS],
                                in0=e32[:, blk * NS:(blk + 1) * NS],
                                in1=rinv_b[:], op=Alu.mult)
                if use_fastr:
                    # one cheap reciprocal on the unique rows, then bf16
                    rinv = setup.tile([T, NS], fp32, tag="rinv")
                    nc.vector.reciprocal(rinv[:], racc[:])
                    rinv16 = setup.tile([T, NS], bf16, tag="rinv16")
                    nc.vector.tensor_copy(rinv16[:], rinv[:])
                    # broadcast back via a DRAM bounce (same SP DMA queue
                    # keeps write->read ordering)
                    nc.sync.dma_start(rr16_d[:], rinv16[:])
                    rrb = setup.tile([128, NBLK * NS], bf16, tag="rrb")
                    rv3 = rr16_d[:].rearrange("(a b) n -> a b n", b=4)
                    for pg in range(4):
                        s = rv3[:, pg, :]
                        s2 = s.unsqueeze(0).broadcast_to([32, NBLK, NS])
                        nc.sync.dma_start(
                            rrb[pg * 32:(pg + 1) * 32, :].rearrange(
                                "p (b n) -> p b n", b=NBLK), s2)
                    nc.vector.tensor_tensor(out=theta[:], in0=e32[:],
                                            in1=rrb[:], op=Alu.mult)

                # phibar = 1 - sigmoid(phi)
                phis = setup.tile([128, NBLK * D], bf16, tag="phis")
                nc.scalar.activation(phis[:], phi32[:], Act.Sigmoid)
                nc.vector.tensor_scalar(out=phibar[:], in0=phis[:],
                                        scalar1=-1.0, scalar2=1.0,
                                        op0=Alu.mult, op1=Alu.add)

            # ===== data-loss main loop =====
            with (
                tc.tile_pool(name="big", bufs=1) as big,
                tc.tile_pool(name="pi4p", bufs=2,
                             space=bass.MemorySpace.PSUM) as pi4p,
            ):
                for c in range(DC):
                    efc = efT[:, c * NS:(c + 1) * NS]
                    L1f = big.tile([128, T * NS], bf16, tag="L1f")
                    if not use_masktt:
                        bm = big.tile([128, T * NS], bf16, tag="bm")
                        nc.sync.dma_start(bm[:],
                                          b_d[:, c * T * NS:(c + 1) * T * NS])
                    phA = big.tile([128, 4 * NS], bf16, tag="phA")
                    nc.vector.memset(phA[:], 0.0)
                    phP = big.tile([128, 4 * NS], bf16, tag="phP")
                    nc.vector.memset(phP[:], 0.0)

                    g0 = 0
                    for h in range(2):
                        ng = HALF_G[h]
                        for g in range(g0, g0 + ng):
                            pi4 = pi4p.tile([128, 4 * 512], fp32, tag="pi4")
                            for j in range(4):
                                t = 4 * g + j
                                blk, prow = t // 4, 32 * (t % 4)
                                nc.tensor.matmul(
                                    pi4[:, j * 512:j * 512 + NS],
                                    phibar[prow:prow + KP,
                                           blk * D + c * 128:
                                           blk * D + c * 128 + 128],
                                    theta[prow:prow + KP,
                                          blk * NS:(blk + 1) * NS],
                                    skip_group_check=True,
                                    tile_position=(prow, 0))
                            pi4v = pi4[:, :].rearrange(
                                "p (t q) -> p t q", q=512)[:, :, 0:NS]
                            nc.scalar.activation(
                                L1f[:, g * 4 * NS:(g + 1) * 4 * NS].rearrange(
                                    "p (t n) -> p t n", t=4),
                                pi4v, Act.Ln)

                        lo, hi = g0 * 4 * NS, (g0 + ng) * 4 * NS
                        nt = ng * 4
                        if use_masktt:
                            moff = c * 2 * T * NS + (0 if h == 0 else
                                                     2 * HALF_G[0] * 4 * NS)
                            hlen = nt * NS
                            am = big.tile([128, HALF_G[0] * 4 * NS], bf16,
                                          tag="mask", bufs=2)
                            nc.sync.dma_start(am[:, 0:hlen],
                                              masks_d[:, moff:moff + hlen])
                            nc.vector.tensor_tensor(out=am[:, 0:hlen],
                                                    in0=am[:, 0:hlen],
                                                    in1=L1f[:, lo:hi],
                                                    op=Alu.mult)
                            for g in range(g0, g0 + ng):
                                s0_ = (g - g0) * 4 * NS
                                nc.vector.tensor_tensor(
                                    out=phA[:], in0=phA[:],
                                    in1=am[:, s0_:s0_ + 4 * NS], op=Alu.add)
                            bm2 = big.tile([128, HALF_G[0] * 4 * NS], bf16,
                                           tag="mask", bufs=2)
                            nc.sync.dma_start(
                                bm2[:, 0:hlen],
                                masks_d[:, moff + hlen:moff + 2 * hlen])
                            nc.vector.tensor_tensor(out=bm2[:, 0:hlen],
                                                    in0=bm2[:, 0:hlen],
                                                    in1=L1f[:, lo:hi],
                                                    op=Alu.mult)
                            for g in range(g0, g0 + ng):
                                s0_ = (g - g0) * 4 * NS
                                nc.vector.tensor_tensor(
                                    out=phP[:], in0=phP[:],
                                    in1=bm2[:, s0_:s0_ + 4 * NS], op=Alu.add)
                            g0 += ng
                            continue
                        # p = b * L1  (in place over the mask tile)
                        nc.vector.tensor_tensor(out=bm[:, lo:hi],
                                                in0=bm[:, lo:hi],
                                                in1=L1f[:, lo:hi], op=Alu.mult)
                        if use_paged:
                            # psi = (t <= ef) * L1  (in place over L1f)
                            lv = L1f[:, lo:hi].rearrange(
                                "p (t n) -> p t n", t=nt)
                            mo = efc.unsqueeze(1).broadcast_to([128, nt, NS])
                            nc.vector.tensor_paged_mask(
                                out=lv, in_=lv,
                                partition_indices=float(4 * g0 - 1),
                                partition_step=1.0, mask_offsets=mo)
                        else:
                            for t in range(4 * g0, 4 * (g0 + ng)):
                                sl_ = slice(t * NS, (t + 1) * NS)
                                nc.vector.scalar_tensor_tensor(
                                    out=L1f[:, sl_], in0=efc,
                                    scalar=float(t), in1=L1f[:, sl_],
                                    op0=Alu.is_ge, op1=Alu.mult)
                        # phase folds
                        for g in range(g0, g0 + ng):
                            s = slice(g * 4 * NS, (g + 1) * 4 * NS)
                            nc.vector.tensor_tensor(out=phA[:], in0=phA[:],
                                                    in1=L1f[:, s], op=Alu.add)
                            nc.vector.tensor_tensor(out=phP[:], in0=phP[:],
                                                    in1=bm[:, s], op=Alu.add)
                        g0 += ng

                    # ---- finals: fold 4 phases -> [128, NS] ----
                    psA = cpx.tile([128, NS], bf16, tag="psA")
                    nc.vector.tensor_tensor(out=psA[:], in0=phA[:, 0:NS],
                                            in1=phA[:, NS:2 * NS], op=Alu.add)
                    nc.vector.tensor_tensor(out=psA[:], in0=psA[:],
                                            in1=phA[:, 2 * NS:3 * NS],
                                            op=Alu.add)
                    nc.vector.tensor_tensor(out=psA[:], in0=psA[:],
                                            in1=phA[:, 3 * NS:4 * NS],
                                            op=Alu.add)
                    ce = cpx.tile([128, NS], bf16, tag="ce")
                    nc.vector.tensor_tensor(out=ce[:], in0=phP[:, 0:NS],
                                            in1=phP[:, NS:2 * NS], op=Alu.add)
                    nc.vector.tensor_tensor(out=ce[:], in0=ce[:],
                                            in1=phP[:, 2 * NS:3 * NS],
                                            op=Alu.add)
                    nc.vector.tensor_tensor(out=ce[:], in0=ce[:],
                                            in1=phP[:, 3 * NS:4 * NS],
                                            op=Alu.add)
                    # dacc col c = sum_n sum_t psi
                    nc.vector.tensor_reduce(out=dacc[:, c:c + 1], in_=psA[:],
                                            axis=mybir.AxisListType.X,
                                            op=Alu.add)
                    # ---- event correction ----
                    X = cpx.tile([128, NS], fp32, tag="X")
                    nc.scalar.activation(X[:], ce[:], Act.Exp)
                    yec = yeT[:, c * NS:(c + 1) * NS]
                    gt_ = cpx.tile([128, NS], fp32, tag="g")
                    nc.vector.tensor_tensor(out=gt_[:], in0=X[:], in1=yec,
                                            op=Alu.add)
                    nc.vector.tensor_scalar(out=gt_[:], in0=gt_[:],
                                            scalar1=-1.0, scalar2=2.0,
                                            op0=Alu.mult, op1=Alu.add)
                    nc.vector.tensor_scalar(out=gt_[:], in0=gt_[:],
                                            scalar1=1e-9, scalar2=None,
                                            op0=Alu.max)
                    lnG = cpx.tile([128, NS], fp32, tag="lnG")
                    nc.scalar.activation(lnG[:], gt_[:], Act.Ln)
                    nc.vector.tensor_tensor(out=lnG[:], in0=lnG[:], in1=ce[:],
                                            op=Alu.subtract)
                    nc.vector.scalar_tensor_tensor(
                        out=lnG[:], in0=yec, scalar=1.0, in1=lnG[:],
                        op0=Alu.mult, op1=Alu.mult,
                        accum_out=dacc[:, 2 + c: 3 + c])

            nc.sync.dma_start(o_dacc[:], dacc[:])

    if not nc.is_finalized():
        nc.finalize()
    return nc


def _prep_inputs(lambda_, phi, gamma, G, Y, logit_prev_t, event_times):
    lam = np.asarray(lambda_, dtype=np.float32)
    phi = np.asarray(phi, dtype=np.float32)
    gamma = np.asarray(gamma, dtype=np.float32)
    G = np.asarray(G, dtype=np.float32)
    ef = np.asarray(event_times)

    # phi in (t,k)-packed layout [52,32,256] -> [128, 13*256]
    arrp = np.zeros((T, KP, D), np.float32)
    arrp[:, :K, :] = phi.transpose(2, 0, 1)
    phi32 = np.ascontiguousarray(
        arrp.reshape(NBLK, 128, D).transpose(1, 0, 2).reshape(128, NBLK * D)
    ).astype(BF16)

    # fused phi gram input: [phi_row | lp_row] pairs
    prows = phi.reshape(K * D, T)
    lp_rows = np.tile(np.asarray(logit_prev_t, np.float32), (K, 1))
    pair = np.concatenate([prows, lp_rows], axis=1)          # [5120, 104]
    phig2 = np.ascontiguousarray(
        pair.reshape(40, 128, 2 * T).transpose(1, 0, 2).reshape(128, 40 * 2 * T)
    ).astype(BF16)

    gam16 = gamma.astype(BF16)
    tgrid = np.arange(T, dtype=np.float32)

    in_maps = []
    for c in range(M):
        sl = slice(c * NS, (c + 1) * NS)
        lam_c = lam[sl]                       # [500, 20, 52]
        arr = np.full((T, KP, NS), -1e4, np.float32)
        arr[:, :K, :] = lam_c.transpose(2, 1, 0)
        lam32 = np.ascontiguousarray(
            arr.reshape(NBLK, 128, NS).transpose(1, 0, 2)
            .reshape(128, NBLK * NS)).astype(BF16)

        efc = ef[sl].astype(np.float32)       # [500, 256]
        efp = efc.T.reshape(DC, 128, NS)      # [2, 128, 500] (chunk, d, n)
        efT = np.ascontiguousarray(
            efp.transpose(1, 0, 2).reshape(128, DC * NS)).astype(BF16)
        ye = np.take_along_axis(np.asarray(Y[sl], np.float32),
                                ef[sl][:, :, None].astype(np.int64),
                                axis=2)[:, :, 0]
        yeT = np.ascontiguousarray(
            ye.T.reshape(DC, 128, NS).transpose(1, 0, 2).reshape(128, DC * NS)
        ).astype(BF16)

        # one-hot b mask, t-major per chunk: [128, (chunk, t, n)]
        efr = efp.transpose(1, 0, 2)              # [128, chunk, n]
        bm = (efr[:, :, None, :] == tgrid[None, None, :, None])
        b_host = np.ascontiguousarray(
            bm.reshape(128, DC * T * NS)).astype(BF16)
        am_full = (efr[:, :, None, :] >= tgrid[None, None, :, None])
        # interleave [a_h0|b_h0|a_h1|b_h1] per chunk
        h0 = HALF_G[0] * 4
        parts = []
        for cc in range(DC):
            parts += [am_full[:, cc, :h0, :].reshape(128, -1),
                      bm[:, cc, :h0, :].reshape(128, -1),
                      am_full[:, cc, h0:, :].reshape(128, -1),
                      bm[:, cc, h0:, :].reshape(128, -1)]
        masks_host = np.ascontiguousarray(
            np.concatenate(parts, axis=1)).astype(BF16)

        # lam rows with a mean hole: [125, (4*K slots) x 53]
        lamr = lam_c.reshape(4, 125, K, T).transpose(1, 0, 2, 3)  # [125,4,20,52]
        lamh = np.zeros((125, 4, K, T + 1), np.float32)
        lamh[:, :, :, :T] = lamr
        lamg = np.ascontiguousarray(
            lamh.reshape(125, 4 * K * (T + 1))).astype(BF16)
        gtg = np.concatenate([G[sl].T.astype(BF16), gam16], axis=1)

        im = dict(
            lam32=lam32, phi32=phi32, efT=efT, yeT=yeT, lamg=lamg,
            gtg=np.ascontiguousarray(gtg), phig2=phig2, bmask=b_host,
        )
        import os as _os
        if _os.environ.get("KMASKTT", "0") == "1":
            im["masks"] = masks_host
        in_maps.append(im)
    return in_maps


def kernel(lambda_, phi, gamma, G, Y, logit_prev_t, event_times):
    from concourse.bass_utils import run_bass_kernel_spmd

    if "nc" not in _COMPILED:
        _COMPILED["nc"] = _build_nc()
    nc = _COMPILED["nc"]

    in_maps = _prep_inputs(lambda_, phi, gamma, G, Y, logit_prev_t, event_times)
    res = run_bass_kernel_spmd(nc, in_maps, list(range(M)))

    data_sum = 0.0
    q_lam = 0.0
    for c in range(M):
        r = res.results[c]
        data_sum += float(r["o_dacc"].astype(np.float64).sum())
        g = r["o_glam"].astype(np.float64)
        A = g[0:T, 0:T]
        b = g[T, 0:T].reshape(T, 1)
        m2 = g[T, T]
        ones = np.ones((T, 1))
        S = A - b @ ones.T - ones @ b.T + m2
        q_lam += float((_KINV_LAM * S).sum())
    gp = res.results[0]["o_gphi"].astype(np.float64)
    Ap = gp[0:T, 0:T]
    Bp = gp[T:2 * T, 0:T]
    Cp = gp[T:2 * T, T:2 * T]
    Sp = Ap - Bp - Bp.T + Cp
    q_phi = float((_KINV_PHI * Sp).sum())

    loss = -data_sum / N + 0.5 * q_lam / N + 0.5 * q_phi / D
    return np.array(loss, dtype=np.float32)
